# revision 17
# baseline (speedup 1.0000x reference)
"""AdaptiveRoutingLayer kernel for 8 TRN2 NeuronCores.

Math: out = sum_e softmax(routing_weights[task_id])[e] * (x @ W[e].T + b[e])
The weighted sum over experts is linear, so it collapses to a single matmul:
    out = x @ Wmix.T + bmix,  Wmix = sum_e w[e] * W[e],  bmix = sum_e w[e] * b[e]
Host mixes the weights (cheap: E*D*D MACs); the device does the B x D x D
matmul, data-parallel over the 8 cores (1024 tokens each). No collectives.

Precision split along the contraction dim (rel-err budget 2e-2, measured
1.60e-2 in fp64 simulation of exactly this quantization):
  k-tiles  0..11 (1536 rows): bf16 x  @ bf16 (64*Wmix)
  k-tiles 12..15 ( 512 rows): fp8e4m3 x @ fp8e4m3 (64*Wmix), DoubleRow pairs
Everything accumulates into one fp32 PSUM chain at scale 64; eviction does
(psum + 64*bias) on the DVE, then *1/64 + bf16 downcast on the scalar engine.
"""

import numpy as np
import ml_dtypes

# Problem shapes (hardcoded; kernel.py must be self-contained).
E, T, D, B = 8, 4, 2048, 8192
N_CORES = 8
B_SH = B // N_CORES          # 1024 tokens per core
P = 128                      # SBUF partitions
KT = 12                      # bf16 k-tiles of 128 (rows 0..1535)
NPAIR = 2                    # fp8 DoubleRow pairs (rows 1536..2047)
KF = KT * P                  # 1536: first fp8 row
NTILE = 512                  # matmul free dim (one PSUM bank of fp32)
HD = D // 2                  # 1024: column half of the output / W
HB = B_SH // 2               # 512 tokens (a/b halves)
S = 64.0                     # product scale of the accumulation

_CACHE = {}


def _build():
    """Build + compile the per-core Bass/Tile graph (same program on all 8 cores)."""
    import concourse.bacc as bacc
    import concourse.mybir as mybir
    import concourse.tile as tile

    nc = bacc.Bacc("TRN2", target_bir_lowering=False, debug=False,
                   num_devices=N_CORES)

    bf16 = mybir.dt.bfloat16
    f32 = mybir.dt.float32
    f8 = mybir.dt.float8e4
    DR = mybir.MatmulPerfMode.DoubleRow
    Copy = mybir.ActivationFunctionType.Copy

    xT = nc.dram_tensor("xT", [KF, B_SH], bf16, kind="ExternalInput").ap()
    # [p, pair, plane, tok]: x.T row KF + (2*pair+plane)*128 + p, fp8
    x8a = nc.dram_tensor("x8a", [P, NPAIR, 2, HB], f8, kind="ExternalInput").ap()
    x8b = nc.dram_tensor("x8b", [P, NPAIR, 2, HB], f8, kind="ExternalInput").ap()
    wT = nc.dram_tensor("wT", [KF, D], bf16, kind="ExternalInput").ap()
    # [p, pair, plane, o-half]: (64*Wmix.T) row KF + (2*pair+plane)*128 + p
    w8_0 = nc.dram_tensor("w8_0", [P, NPAIR, 2, HD], f8, kind="ExternalInput").ap()
    w8_1 = nc.dram_tensor("w8_1", [P, NPAIR, 2, HD], f8, kind="ExternalInput").ap()
    bias = nc.dram_tensor("bias", [P, D], bf16, kind="ExternalInput").ap()  # 64*bmix
    out = nc.dram_tensor("out", [B_SH, D], bf16, kind="ExternalOutput").ap()

    with tile.TileContext(nc) as tc:
        with (
            tc.tile_pool(name="wpool", bufs=1) as wpool,
            tc.tile_pool(name="xpool", bufs=1) as xpool,
            tc.tile_pool(name="bpool", bufs=1) as bpool,
            tc.tile_pool(name="fpool", bufs=2) as fpool,
            tc.tile_pool(name="opool", bufs=4) as opool,
            tc.tile_pool(name="pspool", bufs=1, space="PSUM") as pspool,
        ):
            # Whole working set is SBUF-resident. Separate tiles per k-tile so
            # the PE can start as each DMA lands. x split by token half: pass 1
            # (m 0-3) only needs xa, keeping pass-1 DMA demand under the PE's
            # consumption rate.
            xa_tiles = [xpool.tile([P, HB], bf16, name=f"xa{kt}", tag=f"xa{kt}")
                        for kt in range(KT)]
            xb_tiles = [xpool.tile([P, HB], bf16, name=f"xb{kt}", tag=f"xb{kt}")
                        for kt in range(KT)]
            x8a_t = xpool.tile([P, NPAIR, 2, HB], f8, name="x8a_t", tag="f8xa")
            x8b_t = xpool.tile([P, NPAIR, 2, HB], f8, name="x8b_t", tag="f8xb")
            w_tiles = {}
            for h in range(2):
                for kt in range(KT):
                    w_tiles[(kt, h)] = wpool.tile(
                        [P, HD], bf16, name=f"w{kt}_{h}", tag=f"w{kt}_{h}")
            w8_t = [wpool.tile([P, NPAIR, 2, HD], f8, name=f"w8t_{h}", tag=f"f8w{h}")
                    for h in range(2)]
            b_s = bpool.tile([P, D], bf16)

            # DMA order = consumption order: (x-first-half, w-half-0) per
            # k-tile first, then the fp8 tail tiles (consumed at the end of
            # each chain), then pass-2..4 tiles. x goes through the sync
            # queue, W through the gpsimd queue: two hardware DMA rings fill
            # in parallel, halving both issue pressure and time-to-first-tile.
            # kt0 is split into small pieces so the first matmul's deps (32KB
            # of x + 128KB of W) land ~1us after the ring starts moving.
            for q in range(4):
                nc.sync.dma_start(xa_tiles[0][:, q * P:(q + 1) * P],
                                  xT[0:P, q * P:(q + 1) * P])
                if q < 2:
                    nc.sync.dma_start(
                        w_tiles[(0, 0)][:, q * NTILE:(q + 1) * NTILE],
                        wT[0:P, q * NTILE:(q + 1) * NTILE])
            for kt in range(1, KT):
                nc.sync.dma_start(xa_tiles[kt][:], xT[kt * P:(kt + 1) * P, 0:HB])
                nc.sync.dma_start(w_tiles[(kt, 0)][:], wT[kt * P:(kt + 1) * P, 0:HD])
                if kt == 5:  # early enough for pass-1 evictions (~40us in),
                    nc.sync.dma_start(b_s[:], bias[:])  # late enough not to
                    # stall the pass-1 k-tile stream while the PE is cold
            nc.sync.dma_start(x8a_t[:], x8a)
            nc.sync.dma_start(w8_t[0][:], w8_0)
            for kt in range(KT):
                nc.sync.dma_start(xb_tiles[kt][:], xT[kt * P:(kt + 1) * P, HB:B_SH])
            nc.sync.dma_start(x8b_t[:], x8b)
            for kt in range(KT):
                nc.sync.dma_start(w_tiles[(kt, 1)][:], wT[kt * P:(kt + 1) * P, HD:D])
            nc.sync.dma_start(w8_t[1][:], w8_1)

            # PE warm-up: small dummy matmuls with no DMA deps fill the
            # otherwise idle window until the first k-tiles land (~12.4us:
            # DMA ring start latency + ramping transfer rate), keeping the
            # HAM activity window busy so the real stream starts at 2.4 GHz.
            # N=128 granularity (~107ns cold) minimizes the overshoot.
            warm = bpool.tile([P, NTILE], bf16, name="warm")
            nc.vector.memset(warm[:], 0.0)

            # 4 passes x (4 m-tiles x 1024 cols); all 8 PSUM banks live per
            # pass. Chain steps per (pass, m): 12 bf16 k-tiles + 2 fp8 pairs.
            NSTEP = KT + NPAIR
            # Chain 3 of the last pass finishes the whole kernel: regroup its
            # last 3 k-tiles column-major so its column regions complete (and
            # evict) in stages, ending on a 256-col region whose eviction +
            # out-DMA are all that remain after the last matmul.
            CH3_TAIL = {KT - 3: [(KT - 3, 0, 512)],
                        KT - 2: [(KT - 2, 0, 512), (KT - 1, 0, 512),
                                 (KT - 3, 2, 256), (KT - 3, 3, 256)],
                        KT - 1: [(KT - 2, 2, 256), (KT - 1, 2, 256),
                                 (KT - 2, 3, 256), (KT - 1, 3, 256)]}
            first = True
            for mg, h in ((0, 0), (1, 0), (0, 1), (1, 1)):
                ps = [pspool.tile([P, HD], f32, name=f"ps{mg}{h}{i}", tag=f"ps{i}")
                      for i in range(4)]
                if first:
                    first = False
                    for _ in range(48):
                        nc.tensor.matmul(ps[0][:, 0:P], warm[:, 0:P],
                                         warm[:, 0:P], start=True, stop=True)
                last_pass = (mg, h) == (1, 1)
                if last_pass:
                    # fp8 pairs first (steady block, LDWEIGHTS hidden), then
                    # the bf16 chains staggered so chains 0-2 stop well before
                    # the stream end: their evictions and out-DMAs overlap the
                    # remaining matmuls.
                    delta = (0, 3, 6, 9)
                    sched = [(i, NSTEP - 1 - st) for st in range(NPAIR)
                             for i in range(4)]
                    sched += [(i, v - delta[i])
                              for v in range(KT + delta[-1]) for i in range(4)
                              if 0 <= v - delta[i] < KT]
                else:
                    sched = [(i, st) for st in range(NSTEP) for i in range(4)]
                xh = xa_tiles if mg == 0 else xb_tiles
                x8 = x8a_t if mg == 0 else x8b_t
                for i, st in sched:
                    if st < KT:
                        if last_pass and i == 3 and st in CH3_TAIL:
                            work = CH3_TAIL[st]
                        else:
                            work = [(st, 0, 512), (st, 2, 512)]
                        for kt, q, wd in work:
                            lhsT = xh[kt][:, i * P:(i + 1) * P]  # [K=128, M=128]
                            nc.tensor.matmul(
                                ps[i][:, q * 256:q * 256 + wd],
                                lhsT,
                                w_tiles[(kt, h)][:, q * 256:q * 256 + wd],
                                start=(kt == 0 and not last_pass),
                                stop=(last_pass and kt == KT - 1),
                            )
                    else:
                        pr = st - KT
                        lhsT = x8[:, pr, :, i * P:(i + 1) * P]  # [128, 2, 128]
                        for n2 in range(2):
                            nc.tensor.matmul(
                                ps[i][:, n2 * NTILE:(n2 + 1) * NTILE],
                                lhsT,
                                w8_t[h][:, pr, :, n2 * NTILE:(n2 + 1) * NTILE],
                                start=(last_pass and pr == NPAIR - 1),
                                stop=(not last_pass and pr == NPAIR - 1),
                                perf_mode=DR,
                            )
                for i in range(4):
                    m = mg * 4 + i
                    t32 = fpool.tile([P, HD], f32, name=f"t{mg}{h}{i}", tag=f"t{i}")
                    o_t = opool.tile([P, HD], bf16, name=f"o{mg}{h}{i}", tag="o")
                    if last_pass and i == 3:
                        # chunk the very last eviction to match CH3_TAIL's
                        # staged region completion; the kernel ends on a
                        # 256-col add + scale + out-DMA
                        for c0, wd in ((0, 512), (512, 256), (768, 256)):
                            sl = slice(c0, c0 + wd)
                            gl = slice(h * HD + c0, h * HD + c0 + wd)
                            nc.vector.tensor_add(t32[:, sl], ps[i][:, sl],
                                                 b_s[:, gl])
                            nc.scalar.activation(o_t[:, sl], t32[:, sl],
                                                 Copy, scale=1.0 / S)
                            nc.sync.dma_start(out[m * P:(m + 1) * P, gl],
                                              o_t[:, sl])
                    else:
                        nc.vector.tensor_add(t32[:], ps[i][:],
                                             b_s[:, h * HD:(h + 1) * HD])
                        nc.scalar.activation(o_t[:], t32[:], Copy, scale=1.0 / S)
                        nc.sync.dma_start(
                            out[m * P:(m + 1) * P, h * HD:(h + 1) * HD], o_t[:])

    nc.compile()
    return nc


def _prep_inputs(x, W, b, routing_weights, task_id):
    """Host-side prep: softmax-mix the experts, transpose/quantize, shard."""
    tid = int(np.asarray(task_id))
    r = np.asarray(routing_weights, np.float64)[tid]
    w = np.exp(r - r.max())
    w = (w / w.sum()).astype(np.float32)                 # [E]

    Wmix = np.tensordot(w, np.asarray(W, np.float32), axes=([0], [0]))  # [Dout, Din]
    WmixT = np.ascontiguousarray(Wmix.T) * np.float32(S)                # [Din, Dout]
    bmix = (w[:, None] * np.asarray(b, np.float32)).sum(0) * np.float32(S)
    bias = np.ascontiguousarray(np.broadcast_to(bmix, (P, D))).astype(ml_dtypes.bfloat16)

    xT = np.ascontiguousarray(np.asarray(x, np.float32).T)              # [D, B]
    xT_bf = xT[:KF].astype(ml_dtypes.bfloat16)                          # [1536, B]
    wT_bf = WmixT[:KF].astype(ml_dtypes.bfloat16)                       # [1536, D]

    # fp8 tail: [p, pair, plane, col] = row KF + (2*pair+plane)*128 + p
    x8 = xT[KF:].astype(ml_dtypes.float8_e4m3)                          # [512, B]
    x8 = np.ascontiguousarray(x8.reshape(NPAIR, 2, P, B).transpose(2, 0, 1, 3))
    w8 = WmixT[KF:].astype(ml_dtypes.float8_e4m3)                       # [512, D]
    w8 = np.ascontiguousarray(w8.reshape(NPAIR, 2, P, D).transpose(2, 0, 1, 3))

    in_maps = []
    for c in range(N_CORES):
        t0, t1 = c * B_SH, (c + 1) * B_SH
        in_maps.append({
            "xT": np.ascontiguousarray(xT_bf[:, t0:t1]),
            "x8a": np.ascontiguousarray(x8[:, :, :, t0:t0 + HB]),
            "x8b": np.ascontiguousarray(x8[:, :, :, t0 + HB:t1]),
            "wT": wT_bf,
            "w8_0": np.ascontiguousarray(w8[:, :, :, 0:HD]),
            "w8_1": np.ascontiguousarray(w8[:, :, :, HD:D]),
            "bias": bias,
        })
    return in_maps


def kernel(x, W, b, routing_weights, task_id):
    from concourse.bass_utils import run_bass_kernel_spmd

    in_maps = _prep_inputs(x, W, b, routing_weights, task_id)
    if "nc" not in _CACHE:
        _CACHE["nc"] = _build()
    nc = _CACHE["nc"]

    res = run_bass_kernel_spmd(nc, in_maps, core_ids=list(range(N_CORES)))
    return np.concatenate([res.results[c]["out"] for c in range(N_CORES)],
                          axis=0).astype(np.float32)


# revision 19
# speedup vs baseline: 1.0084x; 1.0084x over previous
"""AdaptiveRoutingLayer kernel for 8 TRN2 NeuronCores.

Math: out = sum_e softmax(routing_weights[task_id])[e] * (x @ W[e].T + b[e])
The weighted sum over experts is linear, so it collapses to a single matmul:
    out = x @ Wmix.T + bmix,  Wmix = sum_e w[e] * W[e],  bmix = sum_e w[e] * b[e]
Host mixes the weights (cheap: E*D*D MACs); the device does the B x D x D
matmul, data-parallel over the 8 cores (1024 tokens each). No collectives.

Precision split along the contraction dim (rel-err budget 2e-2, measured
1.60e-2 in fp64 simulation of exactly this quantization):
  k-tiles  0..11 (1536 rows): bf16 x  @ bf16 (64*Wmix)
  k-tiles 12..15 ( 512 rows): fp8e4m3 x @ fp8e4m3 (64*Wmix), DoubleRow pairs
Everything accumulates into one fp32 PSUM chain at scale 64; eviction does
(psum + 64*bias) on the DVE, then *1/64 + bf16 downcast on the scalar engine.
"""

import numpy as np
import ml_dtypes

# Problem shapes (hardcoded; kernel.py must be self-contained).
E, T, D, B = 8, 4, 2048, 8192
N_CORES = 8
B_SH = B // N_CORES          # 1024 tokens per core
P = 128                      # SBUF partitions
KT = 12                      # bf16 k-tiles of 128 (rows 0..1535)
NPAIR = 2                    # fp8 DoubleRow pairs (rows 1536..2047)
KF = KT * P                  # 1536: first fp8 row
NTILE = 512                  # matmul free dim (one PSUM bank of fp32)
HD = D // 2                  # 1024: column half of the output / W
HB = B_SH // 2               # 512 tokens (a/b halves)
S = 64.0                     # product scale of the accumulation

_CACHE = {}


def _build():
    """Build + compile the per-core Bass/Tile graph (same program on all 8 cores)."""
    import concourse.bacc as bacc
    import concourse.mybir as mybir
    import concourse.tile as tile

    nc = bacc.Bacc("TRN2", target_bir_lowering=False, debug=False,
                   num_devices=N_CORES)

    bf16 = mybir.dt.bfloat16
    f32 = mybir.dt.float32
    f8 = mybir.dt.float8e4
    DR = mybir.MatmulPerfMode.DoubleRow
    Copy = mybir.ActivationFunctionType.Copy

    xT = nc.dram_tensor("xT", [KF, B_SH], bf16, kind="ExternalInput").ap()
    # [p, pair, plane, tok]: x.T row KF + (2*pair+plane)*128 + p, fp8
    x8a = nc.dram_tensor("x8a", [P, NPAIR, 2, HB], f8, kind="ExternalInput").ap()
    x8b = nc.dram_tensor("x8b", [P, NPAIR, 2, HB], f8, kind="ExternalInput").ap()
    wT = nc.dram_tensor("wT", [KF, D], bf16, kind="ExternalInput").ap()
    # [p, pair, plane, o-half]: (64*Wmix.T) row KF + (2*pair+plane)*128 + p
    w8_0 = nc.dram_tensor("w8_0", [P, NPAIR, 2, HD], f8, kind="ExternalInput").ap()
    w8_1 = nc.dram_tensor("w8_1", [P, NPAIR, 2, HD], f8, kind="ExternalInput").ap()
    bias = nc.dram_tensor("bias", [P, D], bf16, kind="ExternalInput").ap()  # 64*bmix
    out = nc.dram_tensor("out", [B_SH, D], bf16, kind="ExternalOutput").ap()

    with tile.TileContext(nc) as tc:
        with (
            tc.tile_pool(name="wpool", bufs=1) as wpool,
            tc.tile_pool(name="xpool", bufs=1) as xpool,
            tc.tile_pool(name="bpool", bufs=1) as bpool,
            tc.tile_pool(name="fpool", bufs=2) as fpool,
            tc.tile_pool(name="opool", bufs=4) as opool,
            tc.tile_pool(name="pspool", bufs=1, space="PSUM") as pspool,
        ):
            # Whole working set is SBUF-resident. Separate tiles per k-tile so
            # the PE can start as each DMA lands. x split by token half: pass 1
            # (m 0-3) only needs xa, keeping pass-1 DMA demand under the PE's
            # consumption rate.
            xa_tiles = [xpool.tile([P, HB], bf16, name=f"xa{kt}", tag=f"xa{kt}")
                        for kt in range(KT)]
            xb_tiles = [xpool.tile([P, HB], bf16, name=f"xb{kt}", tag=f"xb{kt}")
                        for kt in range(KT)]
            x8a_t = xpool.tile([P, NPAIR, 2, HB], f8, name="x8a_t", tag="f8xa")
            x8b_t = xpool.tile([P, NPAIR, 2, HB], f8, name="x8b_t", tag="f8xb")
            w_tiles = {}
            for h in range(2):
                for kt in range(KT):
                    w_tiles[(kt, h)] = wpool.tile(
                        [P, HD], bf16, name=f"w{kt}_{h}", tag=f"w{kt}_{h}")
            w8_t = [wpool.tile([P, NPAIR, 2, HD], f8, name=f"w8t_{h}", tag=f"f8w{h}")
                    for h in range(2)]
            b_s = bpool.tile([P, D], bf16)

            # DMA order = consumption order: (x-first-half, w-half-0) per
            # k-tile first, then the fp8 tail tiles (consumed at the end of
            # each chain), then pass-2..4 tiles.
            for kt in range(KT):
                nc.sync.dma_start(xa_tiles[kt][:], xT[kt * P:(kt + 1) * P, 0:HB])
                nc.sync.dma_start(w_tiles[(kt, 0)][:], wT[kt * P:(kt + 1) * P, 0:HD])
                if kt == 5:  # early enough for pass-1 evictions (~40us in),
                    nc.sync.dma_start(b_s[:], bias[:])  # late enough not to
                    # stall the pass-1 k-tile stream while the PE is cold
            nc.sync.dma_start(x8a_t[:], x8a)
            nc.sync.dma_start(w8_t[0][:], w8_0)
            for kt in range(KT):
                nc.sync.dma_start(xb_tiles[kt][:], xT[kt * P:(kt + 1) * P, HB:B_SH])
            nc.sync.dma_start(x8b_t[:], x8b)
            for kt in range(KT):
                nc.sync.dma_start(w_tiles[(kt, 1)][:], wT[kt * P:(kt + 1) * P, HD:D])
            nc.sync.dma_start(w8_t[1][:], w8_1)

            # PE warm-up: dummy matmuls with no DMA deps fill the otherwise
            # idle window until the first k-tiles land (~12.4us: DMA ring
            # start latency + ramping transfer rate), keeping the HAM
            # activity window busy so the real stream starts at 2.4 GHz.
            warm = bpool.tile([P, NTILE], bf16, name="warm")
            nc.vector.memset(warm[:], 0.0)

            # 4 passes x (4 m-tiles x 1024 cols); all 8 PSUM banks live per
            # pass. Chain steps per (pass, m): 12 bf16 k-tiles + 2 fp8 pairs.
            NSTEP = KT + NPAIR
            # Chain 3 of the last pass finishes the whole kernel: regroup its
            # last 3 k-tiles column-major so its column regions complete (and
            # evict) in stages, ending on a 256-col region whose eviction +
            # out-DMA are all that remain after the last matmul.
            CH3_TAIL = {KT - 3: [(KT - 3, 0, 512)],
                        KT - 2: [(KT - 2, 0, 512), (KT - 1, 0, 512),
                                 (KT - 3, 2, 256), (KT - 3, 3, 256)],
                        KT - 1: [(KT - 2, 2, 256), (KT - 1, 2, 256),
                                 (KT - 2, 3, 256), (KT - 1, 3, 256)]}
            first = True
            for mg, h in ((0, 0), (1, 0), (0, 1), (1, 1)):
                ps = [pspool.tile([P, HD], f32, name=f"ps{mg}{h}{i}", tag=f"ps{i}")
                      for i in range(4)]
                if first:
                    first = False
                    for _ in range(14):
                        nc.tensor.matmul(ps[0][:, 0:NTILE], warm[:, 0:P],
                                         warm[:], start=True, stop=True)
                last_pass = (mg, h) == (1, 1)
                if last_pass:
                    # fp8 pairs first (steady block, LDWEIGHTS hidden), then
                    # the bf16 chains staggered so chains 0-2 stop well before
                    # the stream end: their evictions and out-DMAs overlap the
                    # remaining matmuls.
                    delta = (0, 3, 6, 9)
                    sched = [(i, NSTEP - 1 - st) for st in range(NPAIR)
                             for i in range(4)]
                    sched += [(i, v - delta[i])
                              for v in range(KT + delta[-1]) for i in range(4)
                              if 0 <= v - delta[i] < KT]
                else:
                    sched = [(i, st) for st in range(NSTEP) for i in range(4)]
                xh = xa_tiles if mg == 0 else xb_tiles
                x8 = x8a_t if mg == 0 else x8b_t
                for i, st in sched:
                    if st < KT:
                        if last_pass and i == 3 and st in CH3_TAIL:
                            work = CH3_TAIL[st]
                        else:
                            work = [(st, 0, 512), (st, 2, 512)]
                        for kt, q, wd in work:
                            lhsT = xh[kt][:, i * P:(i + 1) * P]  # [K=128, M=128]
                            nc.tensor.matmul(
                                ps[i][:, q * 256:q * 256 + wd],
                                lhsT,
                                w_tiles[(kt, h)][:, q * 256:q * 256 + wd],
                                start=(kt == 0 and not last_pass),
                                stop=(last_pass and kt == KT - 1),
                            )
                    else:
                        pr = st - KT
                        lhsT = x8[:, pr, :, i * P:(i + 1) * P]  # [128, 2, 128]
                        for n2 in range(2):
                            nc.tensor.matmul(
                                ps[i][:, n2 * NTILE:(n2 + 1) * NTILE],
                                lhsT,
                                w8_t[h][:, pr, :, n2 * NTILE:(n2 + 1) * NTILE],
                                start=(last_pass and pr == NPAIR - 1),
                                stop=(not last_pass and pr == NPAIR - 1),
                                perf_mode=DR,
                            )
                for i in range(4):
                    m = mg * 4 + i
                    t32 = fpool.tile([P, HD], f32, name=f"t{mg}{h}{i}", tag=f"t{i}")
                    o_t = opool.tile([P, HD], bf16, name=f"o{mg}{h}{i}", tag="o")
                    if last_pass and i == 3:
                        # chunk the very last eviction to match CH3_TAIL's
                        # staged region completion; the kernel ends on a
                        # 256-col add + scale + out-DMA
                        for c0, wd in ((0, 512), (512, 256), (768, 256)):
                            sl = slice(c0, c0 + wd)
                            gl = slice(h * HD + c0, h * HD + c0 + wd)
                            nc.vector.tensor_add(t32[:, sl], ps[i][:, sl],
                                                 b_s[:, gl])
                            nc.scalar.activation(o_t[:, sl], t32[:, sl],
                                                 Copy, scale=1.0 / S)
                            nc.sync.dma_start(out[m * P:(m + 1) * P, gl],
                                              o_t[:, sl])
                    else:
                        nc.vector.tensor_add(t32[:], ps[i][:],
                                             b_s[:, h * HD:(h + 1) * HD])
                        nc.scalar.activation(o_t[:], t32[:], Copy, scale=1.0 / S)
                        nc.sync.dma_start(
                            out[m * P:(m + 1) * P, h * HD:(h + 1) * HD], o_t[:])

    nc.compile()
    return nc


def _prep_inputs(x, W, b, routing_weights, task_id):
    """Host-side prep: softmax-mix the experts, transpose/quantize, shard."""
    tid = int(np.asarray(task_id))
    r = np.asarray(routing_weights, np.float64)[tid]
    w = np.exp(r - r.max())
    w = (w / w.sum()).astype(np.float32)                 # [E]

    Wmix = np.tensordot(w, np.asarray(W, np.float32), axes=([0], [0]))  # [Dout, Din]
    WmixT = np.ascontiguousarray(Wmix.T) * np.float32(S)                # [Din, Dout]
    bmix = (w[:, None] * np.asarray(b, np.float32)).sum(0) * np.float32(S)
    bias = np.ascontiguousarray(np.broadcast_to(bmix, (P, D))).astype(ml_dtypes.bfloat16)

    xT = np.ascontiguousarray(np.asarray(x, np.float32).T)              # [D, B]
    xT_bf = xT[:KF].astype(ml_dtypes.bfloat16)                          # [1536, B]
    wT_bf = WmixT[:KF].astype(ml_dtypes.bfloat16)                       # [1536, D]

    # fp8 tail: [p, pair, plane, col] = row KF + (2*pair+plane)*128 + p
    x8 = xT[KF:].astype(ml_dtypes.float8_e4m3)                          # [512, B]
    x8 = np.ascontiguousarray(x8.reshape(NPAIR, 2, P, B).transpose(2, 0, 1, 3))
    w8 = WmixT[KF:].astype(ml_dtypes.float8_e4m3)                       # [512, D]
    w8 = np.ascontiguousarray(w8.reshape(NPAIR, 2, P, D).transpose(2, 0, 1, 3))

    in_maps = []
    for c in range(N_CORES):
        t0, t1 = c * B_SH, (c + 1) * B_SH
        in_maps.append({
            "xT": np.ascontiguousarray(xT_bf[:, t0:t1]),
            "x8a": np.ascontiguousarray(x8[:, :, :, t0:t0 + HB]),
            "x8b": np.ascontiguousarray(x8[:, :, :, t0 + HB:t1]),
            "wT": wT_bf,
            "w8_0": np.ascontiguousarray(w8[:, :, :, 0:HD]),
            "w8_1": np.ascontiguousarray(w8[:, :, :, HD:D]),
            "bias": bias,
        })
    return in_maps


def kernel(x, W, b, routing_weights, task_id):
    from concourse.bass_utils import run_bass_kernel_spmd

    in_maps = _prep_inputs(x, W, b, routing_weights, task_id)
    if "nc" not in _CACHE:
        _CACHE["nc"] = _build()
    nc = _CACHE["nc"]

    res = run_bass_kernel_spmd(nc, in_maps, core_ids=list(range(N_CORES)))
    return np.concatenate([res.results[c]["out"] for c in range(N_CORES)],
                          axis=0).astype(np.float32)


# revision 20
# speedup vs baseline: 1.0153x; 1.0069x over previous
"""AdaptiveRoutingLayer kernel for 8 TRN2 NeuronCores.

Math: out = sum_e softmax(routing_weights[task_id])[e] * (x @ W[e].T + b[e])
The weighted sum over experts is linear, so it collapses to a single matmul:
    out = x @ Wmix.T + bmix,  Wmix = sum_e w[e] * W[e],  bmix = sum_e w[e] * b[e]
Host mixes the weights (cheap: E*D*D MACs); the device does the B x D x D
matmul, data-parallel over the 8 cores (1024 tokens each). No collectives.

Precision split along the contraction dim (rel-err budget 2e-2, measured
1.60e-2 in fp64 simulation of exactly this quantization):
  k-tiles  0..11 (1536 rows): bf16 x  @ bf16 (64*Wmix)
  k-tiles 12..15 ( 512 rows): fp8e4m3 x @ fp8e4m3 (64*Wmix), DoubleRow pairs
Everything accumulates into one fp32 PSUM chain at scale 64; eviction does
(psum + 64*bias) on the DVE, then *1/64 + bf16 downcast on the scalar engine.
"""

import numpy as np
import ml_dtypes

# Problem shapes (hardcoded; kernel.py must be self-contained).
E, T, D, B = 8, 4, 2048, 8192
N_CORES = 8
B_SH = B // N_CORES          # 1024 tokens per core
P = 128                      # SBUF partitions
KT = 12                      # bf16 k-tiles of 128 (rows 0..1535)
NPAIR = 2                    # fp8 DoubleRow pairs (rows 1536..2047)
KF = KT * P                  # 1536: first fp8 row
NTILE = 512                  # matmul free dim (one PSUM bank of fp32)
HD = D // 2                  # 1024: column half of the output / W
HB = B_SH // 2               # 512 tokens (a/b halves)
S = 64.0                     # product scale of the accumulation

_CACHE = {}


def _build():
    """Build + compile the per-core Bass/Tile graph (same program on all 8 cores)."""
    import concourse.bacc as bacc
    import concourse.mybir as mybir
    import concourse.tile as tile

    nc = bacc.Bacc("TRN2", target_bir_lowering=False, debug=False,
                   num_devices=N_CORES)

    bf16 = mybir.dt.bfloat16
    f32 = mybir.dt.float32
    f8 = mybir.dt.float8e4
    DR = mybir.MatmulPerfMode.DoubleRow
    Copy = mybir.ActivationFunctionType.Copy

    xT = nc.dram_tensor("xT", [KF, B_SH], bf16, kind="ExternalInput").ap()
    # [p, pair, plane, tok]: x.T row KF + (2*pair+plane)*128 + p, fp8
    x8a = nc.dram_tensor("x8a", [P, NPAIR, 2, HB], f8, kind="ExternalInput").ap()
    x8b = nc.dram_tensor("x8b", [P, NPAIR, 2, HB], f8, kind="ExternalInput").ap()
    wT = nc.dram_tensor("wT", [KF, D], bf16, kind="ExternalInput").ap()
    # [p, pair, plane, o-half]: (64*Wmix.T) row KF + (2*pair+plane)*128 + p
    w8_0 = nc.dram_tensor("w8_0", [P, NPAIR, 2, HD], f8, kind="ExternalInput").ap()
    w8_1 = nc.dram_tensor("w8_1", [P, NPAIR, 2, HD], f8, kind="ExternalInput").ap()
    bias = nc.dram_tensor("bias", [P, D], bf16, kind="ExternalInput").ap()  # 64*bmix
    out = nc.dram_tensor("out", [B_SH, D], bf16, kind="ExternalOutput").ap()

    with tile.TileContext(nc) as tc:
        with (
            tc.tile_pool(name="wpool", bufs=1) as wpool,
            tc.tile_pool(name="xpool", bufs=1) as xpool,
            tc.tile_pool(name="bpool", bufs=1) as bpool,
            tc.tile_pool(name="fpool", bufs=2) as fpool,
            tc.tile_pool(name="opool", bufs=4) as opool,
            tc.tile_pool(name="pspool", bufs=1, space="PSUM") as pspool,
        ):
            # Whole working set is SBUF-resident. Separate tiles per k-tile so
            # the PE can start as each DMA lands. x split by token half: pass 1
            # (m 0-3) only needs xa, keeping pass-1 DMA demand under the PE's
            # consumption rate.
            xa_tiles = [xpool.tile([P, HB], bf16, name=f"xa{kt}", tag=f"xa{kt}")
                        for kt in range(KT)]
            xb_tiles = [xpool.tile([P, HB], bf16, name=f"xb{kt}", tag=f"xb{kt}")
                        for kt in range(KT)]
            x8a_t = xpool.tile([P, NPAIR, 2, HB], f8, name="x8a_t", tag="f8xa")
            x8b_t = xpool.tile([P, NPAIR, 2, HB], f8, name="x8b_t", tag="f8xb")
            w_tiles = {}
            for h in range(2):
                for kt in range(KT):
                    w_tiles[(kt, h)] = wpool.tile(
                        [P, HD], bf16, name=f"w{kt}_{h}", tag=f"w{kt}_{h}")
            w8_t = [wpool.tile([P, NPAIR, 2, HD], f8, name=f"w8t_{h}", tag=f"f8w{h}")
                    for h in range(2)]
            b_s = bpool.tile([P, D], bf16)

            # DMA order = consumption order: (x-first-half, w-half-0) per
            # k-tile first, then the fp8 tail tiles (consumed at the end of
            # each chain), then pass-2..4 tiles.
            for kt in range(KT):
                nc.sync.dma_start(xa_tiles[kt][:], xT[kt * P:(kt + 1) * P, 0:HB])
                nc.sync.dma_start(w_tiles[(kt, 0)][:], wT[kt * P:(kt + 1) * P, 0:HD])
                if kt == 5:  # early enough for pass-1 evictions (~40us in),
                    nc.sync.dma_start(b_s[:], bias[:])  # late enough not to
                    # stall the pass-1 k-tile stream while the PE is cold
            nc.sync.dma_start(x8a_t[:], x8a)
            nc.sync.dma_start(w8_t[0][:], w8_0)
            for kt in range(KT):
                nc.sync.dma_start(xb_tiles[kt][:], xT[kt * P:(kt + 1) * P, HB:B_SH])
            nc.sync.dma_start(x8b_t[:], x8b)
            for kt in range(KT):
                nc.sync.dma_start(w_tiles[(kt, 1)][:], wT[kt * P:(kt + 1) * P, HD:D])
            nc.sync.dma_start(w8_t[1][:], w8_1)

            # PE warm-up: dummy matmuls with no DMA deps fill the otherwise
            # idle window until the first k-tiles land (~12.4us: DMA ring
            # start latency + ramping transfer rate), keeping the HAM
            # activity window busy so the real stream starts at 2.4 GHz.
            warm = bpool.tile([P, NTILE], bf16, name="warm")
            nc.vector.memset(warm[:], 0.0)

            # 4 passes x (4 m-tiles x 1024 cols); all 8 PSUM banks live per
            # pass. Chain steps per (pass, m): 12 bf16 k-tiles + 2 fp8 pairs.
            NSTEP = KT + NPAIR
            first = True
            for mg, h in ((0, 0), (1, 0), (0, 1), (1, 1)):
                ps = [pspool.tile([P, HD], f32, name=f"ps{mg}{h}{i}", tag=f"ps{i}")
                      for i in range(4)]
                if first:
                    first = False
                    for _ in range(14):
                        nc.tensor.matmul(ps[0][:, 0:NTILE], warm[:, 0:P],
                                         warm[:], start=True, stop=True)
                last_pass = (mg, h) == (1, 1)
                if last_pass:
                    # fp8 pairs first (steady block, LDWEIGHTS hidden), then
                    # the bf16 chains staggered so chains 0-2 stop well before
                    # the stream end: their evictions and out-DMAs overlap the
                    # remaining matmuls.
                    delta = (0, 3, 6, 9)
                    sched = [(i, NSTEP - 1 - st) for st in range(NPAIR)
                             for i in range(4)]
                    sched += [(i, v - delta[i])
                              for v in range(KT + delta[-1]) for i in range(4)
                              if 0 <= v - delta[i] < KT]
                else:
                    sched = [(i, st) for st in range(NSTEP) for i in range(4)]
                xh = xa_tiles if mg == 0 else xb_tiles
                x8 = x8a_t if mg == 0 else x8b_t
                for i, st in sched:
                    if st < KT:
                        kt = st
                        lhsT = xh[kt][:, i * P:(i + 1) * P]   # [K=128, M=128]
                        for n2 in range(2):
                            nc.tensor.matmul(
                                ps[i][:, n2 * NTILE:(n2 + 1) * NTILE],
                                lhsT,
                                w_tiles[(kt, h)][:, n2 * NTILE:(n2 + 1) * NTILE],
                                start=(kt == 0 and not last_pass),
                                stop=(last_pass and kt == KT - 1),
                            )
                    else:
                        pr = st - KT
                        lhsT = x8[:, pr, :, i * P:(i + 1) * P]  # [128, 2, 128]
                        for n2 in range(2):
                            nc.tensor.matmul(
                                ps[i][:, n2 * NTILE:(n2 + 1) * NTILE],
                                lhsT,
                                w8_t[h][:, pr, :, n2 * NTILE:(n2 + 1) * NTILE],
                                start=(last_pass and pr == NPAIR - 1),
                                stop=(not last_pass and pr == NPAIR - 1),
                                perf_mode=DR,
                            )
                for i in range(4):
                    m = mg * 4 + i
                    t32 = fpool.tile([P, HD], f32, name=f"t{mg}{h}{i}", tag=f"t{i}")
                    o_t = opool.tile([P, HD], bf16, name=f"o{mg}{h}{i}", tag="o")
                    if last_pass and i == 3:
                        # chunk the very last eviction: its first out-DMA
                        # overlaps the scale/downcast of the second half
                        for c0, wd in ((0, NTILE), (NTILE, NTILE)):
                            sl = slice(c0, c0 + wd)
                            gl = slice(h * HD + c0, h * HD + c0 + wd)
                            nc.vector.tensor_add(t32[:, sl], ps[i][:, sl],
                                                 b_s[:, gl])
                            nc.scalar.activation(o_t[:, sl], t32[:, sl],
                                                 Copy, scale=1.0 / S)
                            nc.sync.dma_start(out[m * P:(m + 1) * P, gl],
                                              o_t[:, sl])
                    else:
                        nc.vector.tensor_add(t32[:], ps[i][:],
                                             b_s[:, h * HD:(h + 1) * HD])
                        nc.scalar.activation(o_t[:], t32[:], Copy, scale=1.0 / S)
                        nc.sync.dma_start(
                            out[m * P:(m + 1) * P, h * HD:(h + 1) * HD], o_t[:])

    nc.compile()
    return nc


def _prep_inputs(x, W, b, routing_weights, task_id):
    """Host-side prep: softmax-mix the experts, transpose/quantize, shard."""
    tid = int(np.asarray(task_id))
    r = np.asarray(routing_weights, np.float64)[tid]
    w = np.exp(r - r.max())
    w = (w / w.sum()).astype(np.float32)                 # [E]

    Wmix = np.tensordot(w, np.asarray(W, np.float32), axes=([0], [0]))  # [Dout, Din]
    WmixT = np.ascontiguousarray(Wmix.T) * np.float32(S)                # [Din, Dout]
    bmix = (w[:, None] * np.asarray(b, np.float32)).sum(0) * np.float32(S)
    bias = np.ascontiguousarray(np.broadcast_to(bmix, (P, D))).astype(ml_dtypes.bfloat16)

    xT = np.ascontiguousarray(np.asarray(x, np.float32).T)              # [D, B]
    xT_bf = xT[:KF].astype(ml_dtypes.bfloat16)                          # [1536, B]
    wT_bf = WmixT[:KF].astype(ml_dtypes.bfloat16)                       # [1536, D]

    # fp8 tail: [p, pair, plane, col] = row KF + (2*pair+plane)*128 + p
    x8 = xT[KF:].astype(ml_dtypes.float8_e4m3)                          # [512, B]
    x8 = np.ascontiguousarray(x8.reshape(NPAIR, 2, P, B).transpose(2, 0, 1, 3))
    w8 = WmixT[KF:].astype(ml_dtypes.float8_e4m3)                       # [512, D]
    w8 = np.ascontiguousarray(w8.reshape(NPAIR, 2, P, D).transpose(2, 0, 1, 3))

    in_maps = []
    for c in range(N_CORES):
        t0, t1 = c * B_SH, (c + 1) * B_SH
        in_maps.append({
            "xT": np.ascontiguousarray(xT_bf[:, t0:t1]),
            "x8a": np.ascontiguousarray(x8[:, :, :, t0:t0 + HB]),
            "x8b": np.ascontiguousarray(x8[:, :, :, t0 + HB:t1]),
            "wT": wT_bf,
            "w8_0": np.ascontiguousarray(w8[:, :, :, 0:HD]),
            "w8_1": np.ascontiguousarray(w8[:, :, :, HD:D]),
            "bias": bias,
        })
    return in_maps


def kernel(x, W, b, routing_weights, task_id):
    from concourse.bass_utils import run_bass_kernel_spmd

    in_maps = _prep_inputs(x, W, b, routing_weights, task_id)
    if "nc" not in _CACHE:
        _CACHE["nc"] = _build()
    nc = _CACHE["nc"]

    res = run_bass_kernel_spmd(nc, in_maps, core_ids=list(range(N_CORES)))
    return np.concatenate([res.results[c]["out"] for c in range(N_CORES)],
                          axis=0).astype(np.float32)


# revision 23
# speedup vs baseline: 1.0170x; 1.0017x over previous
"""AdaptiveRoutingLayer kernel for 8 TRN2 NeuronCores.

Math: out = sum_e softmax(routing_weights[task_id])[e] * (x @ W[e].T + b[e])
The weighted sum over experts is linear, so it collapses to a single matmul:
    out = x @ Wmix.T + bmix,  Wmix = sum_e w[e] * W[e],  bmix = sum_e w[e] * b[e]
Host mixes the weights (cheap: E*D*D MACs); the device does the B x D x D
matmul, data-parallel over the 8 cores (1024 tokens each). No collectives.

Precision split along the contraction dim (rel-err budget 2e-2, measured
1.60e-2 in fp64 simulation of exactly this quantization):
  k-tiles  0..11 (1536 rows): bf16 x  @ bf16 (64*Wmix)
  k-tiles 12..15 ( 512 rows): fp8e4m3 x @ fp8e4m3 (64*Wmix), DoubleRow pairs
Everything accumulates into one fp32 PSUM chain at scale 64; eviction does
(psum + 64*bias) on the DVE, then *1/64 + bf16 downcast on the scalar engine.
"""

import numpy as np
import ml_dtypes

# Problem shapes (hardcoded; kernel.py must be self-contained).
E, T, D, B = 8, 4, 2048, 8192
N_CORES = 8
B_SH = B // N_CORES          # 1024 tokens per core
P = 128                      # SBUF partitions
KT = 12                      # bf16 k-tiles of 128 (rows 0..1535)
NPAIR = 2                    # fp8 DoubleRow pairs (rows 1536..2047)
KF = KT * P                  # 1536: first fp8 row
NTILE = 512                  # matmul free dim (one PSUM bank of fp32)
HD = D // 2                  # 1024: column half of the output / W
HB = B_SH // 2               # 512 tokens (a/b halves)
S = 64.0                     # product scale of the accumulation

_CACHE = {}


def _build():
    """Build + compile the per-core Bass/Tile graph (same program on all 8 cores)."""
    import concourse.bacc as bacc
    import concourse.mybir as mybir
    import concourse.tile as tile

    nc = bacc.Bacc("TRN2", target_bir_lowering=False, debug=False,
                   num_devices=N_CORES)

    bf16 = mybir.dt.bfloat16
    f32 = mybir.dt.float32
    f8 = mybir.dt.float8e4
    DR = mybir.MatmulPerfMode.DoubleRow
    Copy = mybir.ActivationFunctionType.Copy

    xT = nc.dram_tensor("xT", [KF, B_SH], bf16, kind="ExternalInput").ap()
    # [p, pair, plane, tok]: x.T row KF + (2*pair+plane)*128 + p, fp8
    x8a = nc.dram_tensor("x8a", [P, NPAIR, 2, HB], f8, kind="ExternalInput").ap()
    x8b = nc.dram_tensor("x8b", [P, NPAIR, 2, HB], f8, kind="ExternalInput").ap()
    wT = nc.dram_tensor("wT", [KF, D], bf16, kind="ExternalInput").ap()
    # [p, pair, plane, o-half]: (64*Wmix.T) row KF + (2*pair+plane)*128 + p
    w8_0 = nc.dram_tensor("w8_0", [P, NPAIR, 2, HD], f8, kind="ExternalInput").ap()
    w8_1 = nc.dram_tensor("w8_1", [P, NPAIR, 2, HD], f8, kind="ExternalInput").ap()
    bias = nc.dram_tensor("bias", [P, D], bf16, kind="ExternalInput").ap()  # 64*bmix
    out = nc.dram_tensor("out", [B_SH, D], bf16, kind="ExternalOutput").ap()

    with tile.TileContext(nc) as tc:
        with (
            tc.tile_pool(name="wpool", bufs=1) as wpool,
            tc.tile_pool(name="xpool", bufs=1) as xpool,
            tc.tile_pool(name="bpool", bufs=1) as bpool,
            tc.tile_pool(name="fpool", bufs=2) as fpool,
            tc.tile_pool(name="opool", bufs=4) as opool,
            tc.tile_pool(name="pspool", bufs=1, space="PSUM") as pspool,
        ):
            # Whole working set is SBUF-resident. Separate tiles per k-tile so
            # the PE can start as each DMA lands. x split by token half: pass 1
            # (m 0-3) only needs xa, keeping pass-1 DMA demand under the PE's
            # consumption rate.
            xa_tiles = [xpool.tile([P, HB], bf16, name=f"xa{kt}", tag=f"xa{kt}")
                        for kt in range(KT)]
            xb_tiles = [xpool.tile([P, HB], bf16, name=f"xb{kt}", tag=f"xb{kt}")
                        for kt in range(KT)]
            x8a_t = xpool.tile([P, NPAIR, 2, HB], f8, name="x8a_t", tag="f8xa")
            x8b_t = xpool.tile([P, NPAIR, 2, HB], f8, name="x8b_t", tag="f8xb")
            w_tiles = {}
            for h in range(2):
                for kt in range(KT):
                    w_tiles[(kt, h)] = wpool.tile(
                        [P, HD], bf16, name=f"w{kt}_{h}", tag=f"w{kt}_{h}")
            w8_t = [wpool.tile([P, NPAIR, 2, HD], f8, name=f"w8t_{h}", tag=f"f8w{h}")
                    for h in range(2)]
            b_s = bpool.tile([P, D], bf16)

            # DMA order = consumption order: (x-first-half, w-half-0) per
            # k-tile first, then the fp8 tail tiles (consumed at the end of
            # each chain), then pass-2..4 tiles.
            for kt in range(KT):
                nc.sync.dma_start(xa_tiles[kt][:], xT[kt * P:(kt + 1) * P, 0:HB])
                if kt == 0:
                    # split w00 so the first chain-step's deps (xa0 + 512
                    # cols of W) are ready before the warm bridge ends
                    for n2 in range(2):
                        nc.sync.dma_start(
                            w_tiles[(0, 0)][:, n2 * NTILE:(n2 + 1) * NTILE],
                            wT[0:P, n2 * NTILE:(n2 + 1) * NTILE])
                else:
                    nc.sync.dma_start(w_tiles[(kt, 0)][:],
                                      wT[kt * P:(kt + 1) * P, 0:HD])
                if kt == 5:  # early enough for pass-1 evictions (~40us in),
                    nc.sync.dma_start(b_s[:], bias[:])  # late enough not to
                    # stall the pass-1 k-tile stream while the PE is cold
            nc.sync.dma_start(x8a_t[:], x8a)
            nc.sync.dma_start(w8_t[0][:], w8_0)
            for kt in range(KT):
                nc.sync.dma_start(xb_tiles[kt][:], xT[kt * P:(kt + 1) * P, HB:B_SH])
            nc.sync.dma_start(x8b_t[:], x8b)
            for kt in range(KT):
                nc.sync.dma_start(w_tiles[(kt, 1)][:], wT[kt * P:(kt + 1) * P, HD:D])
            nc.sync.dma_start(w8_t[1][:], w8_1)

            # PE warm-up: dummy matmuls with no DMA deps fill the otherwise
            # idle window until the first k-tiles land (~12.4us: DMA ring
            # start latency + ramping transfer rate), keeping the HAM
            # activity window busy so the real stream starts at 2.4 GHz.
            warm = bpool.tile([P, NTILE], bf16, name="warm")
            nc.vector.memset(warm[:], 0.0)

            # 4 passes x (4 m-tiles x 1024 cols); all 8 PSUM banks live per
            # pass. Chain steps per (pass, m): 12 bf16 k-tiles + 2 fp8 pairs.
            NSTEP = KT + NPAIR
            first = True
            for mg, h in ((0, 0), (1, 0), (0, 1), (1, 1)):
                ps = [pspool.tile([P, HD], f32, name=f"ps{mg}{h}{i}", tag=f"ps{i}")
                      for i in range(4)]
                if first:
                    first = False
                    for _ in range(13):
                        nc.tensor.matmul(ps[0][:, 0:NTILE], warm[:, 0:P],
                                         warm[:], start=True, stop=True)
                last_pass = (mg, h) == (1, 1)
                if last_pass:
                    # fp8 pairs first (steady block, LDWEIGHTS hidden), then
                    # the bf16 chains staggered so chains 0-2 stop well before
                    # the stream end: their evictions and out-DMAs overlap the
                    # remaining matmuls.
                    delta = (0, 3, 6, 9)
                    sched = [(i, NSTEP - 1 - st) for st in range(NPAIR)
                             for i in range(4)]
                    sched += [(i, v - delta[i])
                              for v in range(KT + delta[-1]) for i in range(4)
                              if 0 <= v - delta[i] < KT]
                else:
                    sched = [(i, st) for st in range(NSTEP) for i in range(4)]
                xh = xa_tiles if mg == 0 else xb_tiles
                x8 = x8a_t if mg == 0 else x8b_t
                for i, st in sched:
                    if st < KT:
                        kt = st
                        lhsT = xh[kt][:, i * P:(i + 1) * P]   # [K=128, M=128]
                        for n2 in range(2):
                            nc.tensor.matmul(
                                ps[i][:, n2 * NTILE:(n2 + 1) * NTILE],
                                lhsT,
                                w_tiles[(kt, h)][:, n2 * NTILE:(n2 + 1) * NTILE],
                                start=(kt == 0 and not last_pass),
                                stop=(last_pass and kt == KT - 1),
                            )
                    else:
                        pr = st - KT
                        lhsT = x8[:, pr, :, i * P:(i + 1) * P]  # [128, 2, 128]
                        for n2 in range(2):
                            nc.tensor.matmul(
                                ps[i][:, n2 * NTILE:(n2 + 1) * NTILE],
                                lhsT,
                                w8_t[h][:, pr, :, n2 * NTILE:(n2 + 1) * NTILE],
                                start=(last_pass and pr == NPAIR - 1),
                                stop=(not last_pass and pr == NPAIR - 1),
                                perf_mode=DR,
                            )
                for i in range(4):
                    m = mg * 4 + i
                    t32 = fpool.tile([P, HD], f32, name=f"t{mg}{h}{i}", tag=f"t{i}")
                    o_t = opool.tile([P, HD], bf16, name=f"o{mg}{h}{i}", tag="o")
                    if last_pass and i == 3:
                        # chunk the very last eviction: its first out-DMA
                        # overlaps the scale/downcast of the second half
                        for c0, wd in ((0, NTILE), (NTILE, NTILE)):
                            sl = slice(c0, c0 + wd)
                            gl = slice(h * HD + c0, h * HD + c0 + wd)
                            nc.vector.tensor_add(t32[:, sl], ps[i][:, sl],
                                                 b_s[:, gl])
                            nc.scalar.activation(o_t[:, sl], t32[:, sl],
                                                 Copy, scale=1.0 / S)
                            nc.sync.dma_start(out[m * P:(m + 1) * P, gl],
                                              o_t[:, sl])
                    else:
                        nc.vector.tensor_add(t32[:], ps[i][:],
                                             b_s[:, h * HD:(h + 1) * HD])
                        nc.scalar.activation(o_t[:], t32[:], Copy, scale=1.0 / S)
                        nc.sync.dma_start(
                            out[m * P:(m + 1) * P, h * HD:(h + 1) * HD], o_t[:])

    nc.compile()
    return nc


def _prep_inputs(x, W, b, routing_weights, task_id):
    """Host-side prep: softmax-mix the experts, transpose/quantize, shard."""
    tid = int(np.asarray(task_id))
    r = np.asarray(routing_weights, np.float64)[tid]
    w = np.exp(r - r.max())
    w = (w / w.sum()).astype(np.float32)                 # [E]

    Wmix = np.tensordot(w, np.asarray(W, np.float32), axes=([0], [0]))  # [Dout, Din]
    WmixT = np.ascontiguousarray(Wmix.T) * np.float32(S)                # [Din, Dout]
    bmix = (w[:, None] * np.asarray(b, np.float32)).sum(0) * np.float32(S)
    bias = np.ascontiguousarray(np.broadcast_to(bmix, (P, D))).astype(ml_dtypes.bfloat16)

    xT = np.ascontiguousarray(np.asarray(x, np.float32).T)              # [D, B]
    xT_bf = xT[:KF].astype(ml_dtypes.bfloat16)                          # [1536, B]
    wT_bf = WmixT[:KF].astype(ml_dtypes.bfloat16)                       # [1536, D]

    # fp8 tail: [p, pair, plane, col] = row KF + (2*pair+plane)*128 + p
    x8 = xT[KF:].astype(ml_dtypes.float8_e4m3)                          # [512, B]
    x8 = np.ascontiguousarray(x8.reshape(NPAIR, 2, P, B).transpose(2, 0, 1, 3))
    w8 = WmixT[KF:].astype(ml_dtypes.float8_e4m3)                       # [512, D]
    w8 = np.ascontiguousarray(w8.reshape(NPAIR, 2, P, D).transpose(2, 0, 1, 3))

    in_maps = []
    for c in range(N_CORES):
        t0, t1 = c * B_SH, (c + 1) * B_SH
        in_maps.append({
            "xT": np.ascontiguousarray(xT_bf[:, t0:t1]),
            "x8a": np.ascontiguousarray(x8[:, :, :, t0:t0 + HB]),
            "x8b": np.ascontiguousarray(x8[:, :, :, t0 + HB:t1]),
            "wT": wT_bf,
            "w8_0": np.ascontiguousarray(w8[:, :, :, 0:HD]),
            "w8_1": np.ascontiguousarray(w8[:, :, :, HD:D]),
            "bias": bias,
        })
    return in_maps


def kernel(x, W, b, routing_weights, task_id):
    from concourse.bass_utils import run_bass_kernel_spmd

    in_maps = _prep_inputs(x, W, b, routing_weights, task_id)
    if "nc" not in _CACHE:
        _CACHE["nc"] = _build()
    nc = _CACHE["nc"]

    res = run_bass_kernel_spmd(nc, in_maps, core_ids=list(range(N_CORES)))
    return np.concatenate([res.results[c]["out"] for c in range(N_CORES)],
                          axis=0).astype(np.float32)


# revision 24
# speedup vs baseline: 1.0240x; 1.0069x over previous
"""AdaptiveRoutingLayer kernel for 8 TRN2 NeuronCores.

Math: out = sum_e softmax(routing_weights[task_id])[e] * (x @ W[e].T + b[e])
The weighted sum over experts is linear, so it collapses to a single matmul:
    out = x @ Wmix.T + bmix,  Wmix = sum_e w[e] * W[e],  bmix = sum_e w[e] * b[e]
Host mixes the weights (cheap: E*D*D MACs); the device does the B x D x D
matmul, data-parallel over the 8 cores (1024 tokens each). No collectives.

Precision split along the contraction dim (rel-err budget 2e-2, measured
1.60e-2 in fp64 simulation of exactly this quantization):
  k-tiles  0..11 (1536 rows): bf16 x  @ bf16 (64*Wmix)
  k-tiles 12..15 ( 512 rows): fp8e4m3 x @ fp8e4m3 (64*Wmix), DoubleRow pairs
Everything accumulates into one fp32 PSUM chain at scale 64; eviction does
(psum + 64*bias) on the DVE, then *1/64 + bf16 downcast on the scalar engine.
"""

import numpy as np
import ml_dtypes

# Problem shapes (hardcoded; kernel.py must be self-contained).
E, T, D, B = 8, 4, 2048, 8192
N_CORES = 8
B_SH = B // N_CORES          # 1024 tokens per core
P = 128                      # SBUF partitions
KT = 12                      # bf16 k-tiles of 128 (rows 0..1535)
NPAIR = 2                    # fp8 DoubleRow pairs (rows 1536..2047)
KF = KT * P                  # 1536: first fp8 row
NTILE = 512                  # matmul free dim (one PSUM bank of fp32)
HD = D // 2                  # 1024: column half of the output / W
HB = B_SH // 2               # 512 tokens (a/b halves)
S = 64.0                     # product scale of the accumulation

_CACHE = {}


def _build():
    """Build + compile the per-core Bass/Tile graph (same program on all 8 cores)."""
    import concourse.bacc as bacc
    import concourse.mybir as mybir
    import concourse.tile as tile

    nc = bacc.Bacc("TRN2", target_bir_lowering=False, debug=False,
                   num_devices=N_CORES)

    bf16 = mybir.dt.bfloat16
    f32 = mybir.dt.float32
    f8 = mybir.dt.float8e4
    DR = mybir.MatmulPerfMode.DoubleRow
    Copy = mybir.ActivationFunctionType.Copy

    xT = nc.dram_tensor("xT", [KF, B_SH], bf16, kind="ExternalInput").ap()
    # [p, pair, plane, tok]: x.T row KF + (2*pair+plane)*128 + p, fp8
    x8a = nc.dram_tensor("x8a", [P, NPAIR, 2, HB], f8, kind="ExternalInput").ap()
    x8b = nc.dram_tensor("x8b", [P, NPAIR, 2, HB], f8, kind="ExternalInput").ap()
    wT = nc.dram_tensor("wT", [KF, D], bf16, kind="ExternalInput").ap()
    # [p, pair, plane, o-half]: (64*Wmix.T) row KF + (2*pair+plane)*128 + p
    w8_0 = nc.dram_tensor("w8_0", [P, NPAIR, 2, HD], f8, kind="ExternalInput").ap()
    w8_1 = nc.dram_tensor("w8_1", [P, NPAIR, 2, HD], f8, kind="ExternalInput").ap()
    bias = nc.dram_tensor("bias", [P, D], bf16, kind="ExternalInput").ap()  # 64*bmix
    out = nc.dram_tensor("out", [B_SH, D], bf16, kind="ExternalOutput").ap()

    with tile.TileContext(nc) as tc:
        with (
            tc.tile_pool(name="wpool", bufs=1) as wpool,
            tc.tile_pool(name="xpool", bufs=1) as xpool,
            tc.tile_pool(name="bpool", bufs=1) as bpool,
            tc.tile_pool(name="fpool", bufs=2) as fpool,
            tc.tile_pool(name="opool", bufs=4) as opool,
            tc.tile_pool(name="pspool", bufs=1, space="PSUM") as pspool,
        ):
            # Whole working set is SBUF-resident. Separate tiles per k-tile so
            # the PE can start as each DMA lands. x split by token half: pass 1
            # (m 0-3) only needs xa, keeping pass-1 DMA demand under the PE's
            # consumption rate.
            xa_tiles = [xpool.tile([P, HB], bf16, name=f"xa{kt}", tag=f"xa{kt}")
                        for kt in range(KT)]
            xb_tiles = [xpool.tile([P, HB], bf16, name=f"xb{kt}", tag=f"xb{kt}")
                        for kt in range(KT)]
            x8a_t = xpool.tile([P, NPAIR, 2, HB], f8, name="x8a_t", tag="f8xa")
            x8b_t = xpool.tile([P, NPAIR, 2, HB], f8, name="x8b_t", tag="f8xb")
            w_tiles = {}
            for h in range(2):
                for kt in range(KT):
                    w_tiles[(kt, h)] = wpool.tile(
                        [P, HD], bf16, name=f"w{kt}_{h}", tag=f"w{kt}_{h}")
            w8_t = [wpool.tile([P, NPAIR, 2, HD], f8, name=f"w8t_{h}", tag=f"f8w{h}")
                    for h in range(2)]
            b_s = bpool.tile([P, D], bf16)

            # DMA order = consumption order: (x-first-half, w-half-0) per
            # k-tile first, then the fp8 tail tiles (consumed at the end of
            # each chain), then pass-2..4 tiles.
            for kt in range(KT):
                nc.sync.dma_start(xa_tiles[kt][:], xT[kt * P:(kt + 1) * P, 0:HB])
                if kt == 0:
                    # split w00 so the first chain-step's deps (xa0 + 512
                    # cols of W) are ready before the warm bridge ends
                    for n2 in range(2):
                        nc.sync.dma_start(
                            w_tiles[(0, 0)][:, n2 * NTILE:(n2 + 1) * NTILE],
                            wT[0:P, n2 * NTILE:(n2 + 1) * NTILE])
                else:
                    nc.sync.dma_start(w_tiles[(kt, 0)][:],
                                      wT[kt * P:(kt + 1) * P, 0:HD])
                if kt == 5:  # early enough for pass-1 evictions (~40us in),
                    nc.sync.dma_start(b_s[:], bias[:])  # late enough not to
                    # stall the pass-1 k-tile stream while the PE is cold
            nc.sync.dma_start(x8a_t[:], x8a)
            nc.sync.dma_start(w8_t[0][:], w8_0)
            for kt in range(KT):
                nc.sync.dma_start(xb_tiles[kt][:], xT[kt * P:(kt + 1) * P, HB:B_SH])
            nc.sync.dma_start(x8b_t[:], x8b)
            for kt in range(KT):
                nc.sync.dma_start(w_tiles[(kt, 1)][:], wT[kt * P:(kt + 1) * P, HD:D])
            nc.sync.dma_start(w8_t[1][:], w8_1)

            # PE warm-up: dummy matmuls with no DMA deps fill the otherwise
            # idle window until the first k-tiles land (~12.4us: DMA ring
            # start latency + ramping transfer rate), keeping the HAM
            # activity window busy so the real stream starts at 2.4 GHz.
            warm = bpool.tile([P, NTILE], bf16, name="warm")
            nc.vector.memset(warm[:], 0.0)

            # 4 passes x (4 m-tiles x 1024 cols); all 8 PSUM banks live per
            # pass. Chain steps per (pass, m): 12 bf16 k-tiles + 2 fp8 pairs.
            NSTEP = KT + NPAIR
            first = True
            for mg, h in ((0, 0), (1, 0), (0, 1), (1, 1)):
                ps = [pspool.tile([P, HD], f32, name=f"ps{mg}{h}{i}", tag=f"ps{i}")
                      for i in range(4)]
                if first:
                    first = False
                    for _ in range(10):
                        nc.tensor.matmul(ps[0][:, 0:NTILE], warm[:, 0:P],
                                         warm[:], start=True, stop=True)
                last_pass = (mg, h) == (1, 1)
                if last_pass:
                    # fp8 pairs first (steady block, LDWEIGHTS hidden), then
                    # the bf16 chains staggered so chains 0-2 stop well before
                    # the stream end: their evictions and out-DMAs overlap the
                    # remaining matmuls.
                    delta = (0, 3, 6, 9)
                    sched = [(i, NSTEP - 1 - st) for st in range(NPAIR)
                             for i in range(4)]
                    sched += [(i, v - delta[i])
                              for v in range(KT + delta[-1]) for i in range(4)
                              if 0 <= v - delta[i] < KT]
                else:
                    sched = [(i, st) for st in range(NSTEP) for i in range(4)]
                xh = xa_tiles if mg == 0 else xb_tiles
                x8 = x8a_t if mg == 0 else x8b_t
                for i, st in sched:
                    if st < KT:
                        kt = st
                        lhsT = xh[kt][:, i * P:(i + 1) * P]   # [K=128, M=128]
                        for n2 in range(2):
                            nc.tensor.matmul(
                                ps[i][:, n2 * NTILE:(n2 + 1) * NTILE],
                                lhsT,
                                w_tiles[(kt, h)][:, n2 * NTILE:(n2 + 1) * NTILE],
                                start=(kt == 0 and not last_pass),
                                stop=(last_pass and kt == KT - 1),
                            )
                    else:
                        pr = st - KT
                        lhsT = x8[:, pr, :, i * P:(i + 1) * P]  # [128, 2, 128]
                        for n2 in range(2):
                            nc.tensor.matmul(
                                ps[i][:, n2 * NTILE:(n2 + 1) * NTILE],
                                lhsT,
                                w8_t[h][:, pr, :, n2 * NTILE:(n2 + 1) * NTILE],
                                start=(last_pass and pr == NPAIR - 1),
                                stop=(not last_pass and pr == NPAIR - 1),
                                perf_mode=DR,
                            )
                for i in range(4):
                    m = mg * 4 + i
                    t32 = fpool.tile([P, HD], f32, name=f"t{mg}{h}{i}", tag=f"t{i}")
                    o_t = opool.tile([P, HD], bf16, name=f"o{mg}{h}{i}", tag="o")
                    if last_pass and i == 3:
                        # chunk the very last eviction: its first out-DMA
                        # overlaps the scale/downcast of the second half
                        for c0, wd in ((0, NTILE), (NTILE, NTILE)):
                            sl = slice(c0, c0 + wd)
                            gl = slice(h * HD + c0, h * HD + c0 + wd)
                            nc.vector.tensor_add(t32[:, sl], ps[i][:, sl],
                                                 b_s[:, gl])
                            nc.scalar.activation(o_t[:, sl], t32[:, sl],
                                                 Copy, scale=1.0 / S)
                            nc.sync.dma_start(out[m * P:(m + 1) * P, gl],
                                              o_t[:, sl])
                    else:
                        nc.vector.tensor_add(t32[:], ps[i][:],
                                             b_s[:, h * HD:(h + 1) * HD])
                        nc.scalar.activation(o_t[:], t32[:], Copy, scale=1.0 / S)
                        nc.sync.dma_start(
                            out[m * P:(m + 1) * P, h * HD:(h + 1) * HD], o_t[:])

    nc.compile()
    return nc


def _prep_inputs(x, W, b, routing_weights, task_id):
    """Host-side prep: softmax-mix the experts, transpose/quantize, shard."""
    tid = int(np.asarray(task_id))
    r = np.asarray(routing_weights, np.float64)[tid]
    w = np.exp(r - r.max())
    w = (w / w.sum()).astype(np.float32)                 # [E]

    Wmix = np.tensordot(w, np.asarray(W, np.float32), axes=([0], [0]))  # [Dout, Din]
    WmixT = np.ascontiguousarray(Wmix.T) * np.float32(S)                # [Din, Dout]
    bmix = (w[:, None] * np.asarray(b, np.float32)).sum(0) * np.float32(S)
    bias = np.ascontiguousarray(np.broadcast_to(bmix, (P, D))).astype(ml_dtypes.bfloat16)

    xT = np.ascontiguousarray(np.asarray(x, np.float32).T)              # [D, B]
    xT_bf = xT[:KF].astype(ml_dtypes.bfloat16)                          # [1536, B]
    wT_bf = WmixT[:KF].astype(ml_dtypes.bfloat16)                       # [1536, D]

    # fp8 tail: [p, pair, plane, col] = row KF + (2*pair+plane)*128 + p
    x8 = xT[KF:].astype(ml_dtypes.float8_e4m3)                          # [512, B]
    x8 = np.ascontiguousarray(x8.reshape(NPAIR, 2, P, B).transpose(2, 0, 1, 3))
    w8 = WmixT[KF:].astype(ml_dtypes.float8_e4m3)                       # [512, D]
    w8 = np.ascontiguousarray(w8.reshape(NPAIR, 2, P, D).transpose(2, 0, 1, 3))

    in_maps = []
    for c in range(N_CORES):
        t0, t1 = c * B_SH, (c + 1) * B_SH
        in_maps.append({
            "xT": np.ascontiguousarray(xT_bf[:, t0:t1]),
            "x8a": np.ascontiguousarray(x8[:, :, :, t0:t0 + HB]),
            "x8b": np.ascontiguousarray(x8[:, :, :, t0 + HB:t1]),
            "wT": wT_bf,
            "w8_0": np.ascontiguousarray(w8[:, :, :, 0:HD]),
            "w8_1": np.ascontiguousarray(w8[:, :, :, HD:D]),
            "bias": bias,
        })
    return in_maps


def kernel(x, W, b, routing_weights, task_id):
    from concourse.bass_utils import run_bass_kernel_spmd

    in_maps = _prep_inputs(x, W, b, routing_weights, task_id)
    if "nc" not in _CACHE:
        _CACHE["nc"] = _build()
    nc = _CACHE["nc"]

    res = run_bass_kernel_spmd(nc, in_maps, core_ids=list(range(N_CORES)))
    return np.concatenate([res.results[c]["out"] for c in range(N_CORES)],
                          axis=0).astype(np.float32)
